# revision 21
# baseline (speedup 1.0000x reference)
"""GAT cell (gnn_message_passing) Bass kernel for 8 Trainium2 NeuronCores.

Sharding: pure data parallelism over batch (64 graphs -> 8 per core), both
branches (in/out) on every core.  v3: software-pipelined schedule.

Math per graph/branch, all in transposed (j-partitioned) layout:
  x^T  = Wh^T @ X^T                      [att, N]      (fp16)
  s^T  = x @ (x*a)^T                     [N(j), N(i)]  == score^T
  es   = exp(prelu(s^T))                 (fp16)
  B    = A^T;  T' = A^T | I  (host);  c2 = B @ T' = B + B^2  (fp8 DoubleRow)
  b2s  = sign(c2) = bin(B + B^2)         (fp8 {0,1}, plain ACT Sign)
  b23  = I@b2s + B@b2s   -> >0 iff 1..3-step reachable
  pt   = (b23 > 0) * es                  (fp16)  == P^T unnormalized
  Y    = X @ We, augmented with ones col [N(j), 65]
  U^T  = [Y|1]^T @ P^T                   [65, N(i)]; row 64 = colsums
  host: out = (U / (sum + eps)) + bias   (normalize + bias on host, f32)

v3 perf structure (v2 stalled: PE <1 iteration of lookahead -> HAM cold
42us of 65us):
 - 8 work units (4 pairs x 2 branches); unit k's mask/aggregate phase (B)
   is emitted interleaved with unit k+2's score phase (A), so the PE always
   has independent matmuls between each PE->ACT/DVE->PE round trip.
 - PSUM: tag "small" (1 bank, bufs=2: xt, yo, ut), tag "sc" (2 banks,
   bufs=1), tag "mask" (2 banks, bufs=2: b2, b23) == exactly 8 banks; the
   pipeline skew, not buffer count, provides the lookahead.
 - engine split per unit: ACT prelu+exp+sign(b2) ~2.5us, DVE pt+xt+ys+res
   ~2.4us, GPSIMD xa+memsets, PE ~2.3us -> all co-busy.
 - all DMAs on the sync HWDGE queue; fp8 A / A^T|I, fp16 X^T/weights/out.
"""

import numpy as np
from contextlib import ExitStack

import concourse.bass as bass
import concourse.bacc as bacc
import concourse.tile as tile
from concourse import mybir, bass_utils

F32 = mybir.dt.float32
BF16 = mybir.dt.bfloat16
FP16 = mybir.dt.float16
FP8 = mybir.dt.float8e4
AF = mybir.ActivationFunctionType
ALU = mybir.AluOpType
DR = mybir.MatmulPerfMode.DoubleRow

NCORES = 8
B = 64
BPC = B // NCORES        # graphs per core
PAIRS = BPC // 2         # pairs per core
UNITS = PAIRS * 2        # pair x branch work units
N = 200
H = 256
ATT = 64
EPS = 1e-20
NWARM = 14
LEAD = 2                 # software pipeline skew (units)

USE_DR = True


def _make_identity(nc, identity):
    nc.gpsimd.memset(identity, 0.0)
    nc.gpsimd.affine_select(
        out=identity, in_=identity, compare_op=ALU.not_equal, fill=1.0,
        base=0, pattern=[[-1, 128]], channel_multiplier=1)


def _emit(ctx, tc, AT, XT, WT, AV, O):
    nc = tc.nc
    consts = ctx.enter_context(tc.tile_pool(name="consts", bufs=1))
    pin = ctx.enter_context(tc.tile_pool(name="pin", bufs=4))
    pw = ctx.enter_context(tc.tile_pool(name="pw", bufs=2))
    pp = ctx.enter_context(tc.tile_pool(name="pp", bufs=1, space="PSUM"))

    # ---- DMAs: weights+av on the scalar HWDGE queue; the interleaved
    # px/pa input stream and the outputs on the sync HWDGE queue.
    wt = consts.tile([128, 2, 2, 2, ATT], FP16, tag="wt", name="wt")
    nc.scalar.dma_start(out=wt, in_=WT)
    av = consts.tile([128, 2], F32, tag="av", name="av")
    nc.scalar.dma_start(out=av, in_=AV)
    pxv = [[None, None] for _ in range(PAIRS)]   # [pb][br] -> [128,2,512]
    pa = [None] * PAIRS
    for pb in range(PAIRS):
        px_t = pin.tile([128, 2, 2, 512], FP16, tag="px", bufs=4, name="px_t")
        nc.sync.dma_start(out=px_t, in_=XT[pb])
        pxv[pb][0] = px_t[:, 0]
        pxv[pb][1] = px_t[:, 1]
        pa[pb] = pin.tile([128, 2, 2, 912], FP8, tag="pa", name="pa_t")
        nc.sync.dma_start(out=pa[pb], in_=AT[pb])

    ident = consts.tile([128, 128], BF16, tag="ident", name="ident")
    _make_identity(nc, ident)
    ident8 = consts.tile([128, 128], FP8, tag="ident8", name="ident8")
    nc.vector.tensor_copy(ident8, ident)

    # ---- PE warmup (HAM un-throttle) while the first DMAs stream ----
    wu_ps = pp.tile([128, 128], F32, tag="small", bufs=2, name="wu_ps")
    for _ in range(NWARM):
        nc.tensor.matmul(wu_ps, ident, ident, start=True, stop=True)

    # unit k = (pair, branch); state carried from phase A to phase B
    st = [dict() for _ in range(UNITS)]

    def a_xt(k):
        pb, br = k // 2, k % 2
        s = st[k]
        s["iT"] = [pxv[pb][br][:, i].rearrange("p (c m) -> p c m", c=2)
                   for i in range(2)]
        xt_ps = pp.tile([128, 256], F32, tag="small", bufs=2, name="xt_ps")
        for i in range(2):
            for hc in range(2):
                nc.tensor.matmul(xt_ps[i * 64:(i + 1) * 64, :],
                                 wt[:, br, 0, hc, :], s["iT"][i][:, hc, :],
                                 start=(hc == 0), stop=(hc == 1))
        xt = pw.tile([128, 256], FP16, tag="xt", name="xt")
        nc.vector.tensor_copy(xt, xt_ps)
        xa = pw.tile([128, 256], FP16, tag="xa", name="xa")
        nc.vector.tensor_scalar(out=xa, in0=xt, scalar1=av[:, br:br + 1],
                                scalar2=None, op0=ALU.mult)
        s["xt"], s["xa"] = xt, xa

    def a_yo(k):
        pb, br = k // 2, k % 2
        s = st[k]
        yo_ps = pp.tile([128, 2, 2, ATT + 1], F32, tag="small", bufs=2,
                        name="yo_ps")
        for i in range(2):
            for jc in range(2):
                for hc in range(2):
                    nc.tensor.matmul(
                        yo_ps[:, i, jc, 0:ATT],
                        s["iT"][i][:, hc, jc * 128:(jc + 1) * 128],
                        wt[:, br, 1, hc, :],
                        start=(hc == 0), stop=(hc == 1))
        ys = pw.tile([128, 2, 2, ATT + 1], FP16, tag="ys", bufs=4, name="ys")
        nc.gpsimd.memset(ys[:, :, :, ATT:ATT + 1], 1.0)
        nc.vector.tensor_copy(ys[:, :, :, 0:ATT], yo_ps[:, :, :, 0:ATT])
        s["ys"] = ys

    def a_sc(k):
        s = st[k]
        xt, xa = s["xt"], s["xa"]
        sc_ps = pp.tile([128, 2, 2, 256], F32, tag="sc", bufs=1, name="sc_ps")
        for i in range(2):
            for jc in range(2):
                nc.tensor.matmul(sc_ps[:, i, jc, 0:N],
                                 xt[i * 64:(i + 1) * 64,
                                    jc * 128:(jc + 1) * 128],
                                 xa[i * 64:(i + 1) * 64, 0:N],
                                 start=True, stop=True)
        nc.scalar.activation(out=sc_ps[:, :, :, 0:N], in_=sc_ps[:, :, :, 0:N],
                             func=AF.Prelu, alpha=0.2)
        es = pw.tile([128, 2, 2, N], FP16, tag="es", bufs=4, name="es")
        nc.scalar.activation(out=es, in_=sc_ps[:, :, :, 0:N], func=AF.Exp)
        s["es"] = es

    def b_b2(k):
        pb, br = k // 2, k % 2
        s = st[k]
        s["a0"] = [pa[pb][:, br, i, 0:512].rearrange("p (c m) -> p c m", c=2)
                   for i in range(2)]
        s["T"] = [pa[pb][:, br, i, 512:912].rearrange("p (c m) -> p c m", c=2)
                  for i in range(2)]
        b2_ps = pp.tile([128, 2, 2, 256], F32, tag="mask", bufs=2,
                        name="b2_ps")
        for i in range(2):
            for mc in range(2):
                if USE_DR:
                    nc.tensor.matmul(b2_ps[:, i, mc, 0:N],
                                     s["a0"][i][:, :, mc * 128:(mc + 1) * 128],
                                     s["T"][i], start=True, stop=True,
                                     perf_mode=DR)
                else:
                    for kc in range(2):
                        nc.tensor.matmul(
                            b2_ps[:, i, mc, 0:N],
                            s["a0"][i][:, kc, mc * 128:(mc + 1) * 128],
                            s["T"][i][:, kc, :],
                            start=(kc == 0), stop=(kc == 1))
        b2s = pw.tile([128, 2, 2, N], FP8, tag="b2s", name="b2s")
        nc.scalar.activation(out=b2s, in_=b2_ps[:, :, :, 0:N], func=AF.Sign)
        s["b2s"] = b2s

    def b_b23(k):
        s = st[k]
        b2s = s["b2s"]
        b23_ps = pp.tile([128, 2, 2, 256], F32, tag="mask", bufs=2,
                         name="b23_ps")
        for i in range(2):
            for mc in range(2):
                nc.tensor.matmul(b23_ps[:, i, mc, 0:N], ident8,
                                 b2s[:, i, mc, :], start=True, stop=False)
                if USE_DR:
                    nc.tensor.matmul(b23_ps[:, i, mc, 0:N],
                                     s["a0"][i][:, :, mc * 128:(mc + 1) * 128],
                                     b2s[:, i, :, :], start=False,
                                     stop=True, perf_mode=DR)
                else:
                    for kc in range(2):
                        nc.tensor.matmul(
                            b23_ps[:, i, mc, 0:N],
                            s["a0"][i][:, kc, mc * 128:(mc + 1) * 128],
                            b2s[:, i, kc, :],
                            start=False, stop=(kc == 1))
        s["b23_ps"] = b23_ps

    def b_pt(k):
        s = st[k]
        pt = pw.tile([128, 2, 2, N], FP16, tag="pt", name="pt")
        nc.vector.scalar_tensor_tensor(
            out=pt, in0=s["b23_ps"][:, :, :, 0:N], scalar=0.0, in1=s["es"],
            op0=ALU.is_gt, op1=ALU.mult)
        s["pt"] = pt

    def b_u(k):
        pb, br = k // 2, k % 2
        s = st[k]
        ut_ps = pp.tile([128, 2, N], F32, tag="small", bufs=2, name="ut_ps")
        for i in range(2):
            for jc in range(2):
                nc.tensor.matmul(ut_ps[0:ATT + 1, i, :],
                                 s["ys"][:, i, jc, :], s["pt"][:, i, jc, :],
                                 start=(jc == 0), stop=(jc == 1))
        res = pw.tile([128, 2, N], FP16, tag="res", bufs=4, name="res")
        nc.vector.tensor_copy(res[0:ATT + 1], ut_ps[0:ATT + 1])
        nc.sync.dma_start(out=O[:, br, pb * 2:pb * 2 + 2, :],
                          in_=res[0:ATT + 1])
        st[k] = {}

    def a_phase(k):
        a_xt(k)
        a_yo(k)
        a_sc(k)

    # ---- software-pipelined emission: B(k) interleaved with A(k+LEAD) ----
    for k in range(LEAD):
        a_phase(k)
    for k in range(UNITS):
        b_b2(k)
        if k + LEAD < UNITS:
            a_xt(k + LEAD)
            a_yo(k + LEAD)
        b_b23(k)
        if k + LEAD < UNITS:
            a_sc(k + LEAD)
        b_pt(k)
        b_u(k)


def build(order: int) -> bacc.Bacc:
    assert order == 3, "only order==3 supported"
    nc = bacc.Bacc("TRN2", target_bir_lowering=False, debug=False,
                   enable_asserts=False, num_devices=NCORES)
    AT = nc.dram_tensor("AT", [PAIRS, 128, 2, 2, 912], FP8,
                        kind="ExternalInput").ap()
    XT = nc.dram_tensor("XT", [PAIRS, 128, 2, 2, 512], FP16,
                        kind="ExternalInput").ap()
    WT = nc.dram_tensor("WT", [128, 2, 2, 2, ATT], FP16,
                        kind="ExternalInput").ap()
    AV = nc.dram_tensor("AV", [128, 2], F32, kind="ExternalInput").ap()
    O = nc.dram_tensor("O", [ATT + 1, 2, BPC, N], FP16,
                       kind="ExternalOutput").ap()
    with tile.TileContext(nc) as tc:
        with ExitStack() as ctx:
            _emit(ctx, tc, AT, XT, WT, AV, O)
    nc.compile()
    return nc


_CACHE = {}


def _get(order: int) -> bacc.Bacc:
    if order not in _CACHE:
        _CACHE[order] = build(order)
    return _CACHE[order]


def _fp8():
    import ml_dtypes
    return ml_dtypes.float8_e4m3fn


def _chunk_rows_200(x, cols):
    """[B, 200, cols_src] -> [B, 128, 2, cols] zero-padded row chunks."""
    bsz = x.shape[0]
    out = np.zeros((bsz, 128, 2, cols), np.float32)
    out[:, :, 0, 0:x.shape[2]] = x[:, 0:128, :]
    out[:, 0:72, 1, 0:x.shape[2]] = x[:, 128:200, :]
    return out


def make_in_maps(A_in_0, A_out_0, input_in, input_out,
                 W_head_in, W_head_out, a_in, a_out,
                 W_edge_in, W_edge_out):
    fp8 = _fp8()
    eye = np.eye(N, dtype=np.float32)[None]
    pk_br, xp_br = [], []
    for A, X in ((A_in_0, input_in), (A_out_0, input_out)):
        A = np.asarray(A, np.float32)
        X = np.asarray(X, np.float32)
        a0 = _chunk_rows_200(A, 256)                            # [B,128,2,256]
        # T' = A^T | I so that B @ T' = B + B^2 (sign -> 1..2-step reach)
        tp = np.maximum(np.transpose(A, (0, 2, 1)), eye)
        at = _chunk_rows_200(tp, 200)                           # [B,128,2,200]
        pk = np.concatenate([a0.reshape(B, 128, 512),
                             at.reshape(B, 128, 400)], axis=2)  # [B,128,912]
        pk_br.append(pk.astype(fp8))
        xT = np.transpose(X, (0, 2, 1)).reshape(B, 2, 128, 200)
        xp = np.zeros((B, 128, 2, 256), np.float16)
        xp[:, :, 0, 0:200] = xT[:, 0]
        xp[:, :, 1, 0:200] = xT[:, 1]
        xp_br.append(xp.reshape(B, 128, 512))                   # [B,128,512]

    pk2 = np.stack(pk_br, axis=1)    # [B, 2(br), 128, 912] fp8
    xp2 = np.stack(xp_br, axis=1)    # [B, 2(br), 128, 512] fp16

    wtb = np.zeros((128, 2, 2, 2, ATT), np.float16)
    for bi, (Wh, We) in enumerate(((W_head_in, W_edge_in),
                                   (W_head_out, W_edge_out))):
        Wh = np.asarray(Wh, np.float32)
        We = np.asarray(We, np.float32)
        for hc in range(2):
            wtb[:, bi, 0, hc, :] = Wh[hc * 128:(hc + 1) * 128, :]
            wtb[:, bi, 1, hc, :] = We[hc * 128:(hc + 1) * 128, :]
    avb = np.stack([np.concatenate([a_in, a_in]),
                    np.concatenate([a_out, a_out])],
                   axis=1).astype(np.float32)                   # [128, 2]

    shards = []
    for c in range(NCORES):
        at_c = pk2[c * BPC:(c + 1) * BPC]          # [8, 2, 128, 912]
        at_c = at_c.reshape(PAIRS, 2, 2, 128, 912).transpose(0, 3, 2, 1, 4)
        xt_c = xp2[c * BPC:(c + 1) * BPC]
        xt_c = xt_c.reshape(PAIRS, 2, 2, 128, 512).transpose(0, 3, 2, 1, 4)
        shards.append({
            "AT": np.ascontiguousarray(at_c),
            "XT": np.ascontiguousarray(xt_c),
            "WT": wtb, "AV": avb,
        })
    return shards


def run(trace=False, **inputs):
    order = int(inputs.get("order", 3))
    nc = _get(order)
    in_maps = make_in_maps(
        A_in_0=inputs["A_in_0"], A_out_0=inputs["A_out_0"],
        input_in=inputs["input_in"], input_out=inputs["input_out"],
        W_head_in=inputs["W_head_in"], W_head_out=inputs["W_head_out"],
        a_in=inputs["a_in"], a_out=inputs["a_out"],
        W_edge_in=inputs["W_edge_in"], W_edge_out=inputs["W_edge_out"])
    kw2 = {}
    if trace:
        import os
        td = os.path.join(os.getcwd(), "trace_out")
        os.makedirs(td, exist_ok=True)
        kw2["tmpdir"] = td
    res = bass_utils.run_bass_kernel_spmd(nc, in_maps,
                                          core_ids=list(range(NCORES)),
                                          trace=trace, **kw2)
    bias = {0: np.asarray(inputs["bias_iah"], np.float32),
            1: np.asarray(inputs["bias_oah"], np.float32)}
    outs = {0: [], 1: []}
    for c in range(NCORES):
        o = res.results[c]["O"].astype(np.float32)   # [65, 2, 8, 200]
        for br in range(2):
            u = np.transpose(o[0:ATT, br], (1, 2, 0))      # [8, 200, 64]
            s = o[ATT, br][:, :, None]                     # [8, 200, 1]
            outs[br].append(u / (s + EPS) + bias[br][None, None, :])
    out_in = np.concatenate(outs[0], axis=0).astype(np.float32)
    out_out = np.concatenate(outs[1], axis=0).astype(np.float32)
    return (out_in, out_out), res


def kernel(**inputs):
    (out_in, out_out), _ = run(trace=False, **inputs)
    return out_in, out_out
